# revision 7
# baseline (speedup 1.0000x reference)
"""Causal self-attention Trainium2 kernel — tensor-parallel over heads on 8 NeuronCores.

Problem: B=4, T=2048, C=1024, H=16 heads (head_dim 64), fp32 in/out.
Sharding: 2 heads per core. Each core computes qkv projection for its head
columns, full causal attention for its heads, and a partial output
projection (its W_proj rows); partials are summed on host.

Datapath is bf16 (x, weights, q/k/v, attention weights, y, output partials)
with fp32 PSUM accumulation and fp32 softmax denominators; measured max rel
error ~3e-3 against an fp64 reference. bf16 halves HBM traffic and SBUF
footprint and enables XBAR DMA transpose for the V tiles (no PE transposes).
Engines: PE matmuls only; ACT exp only; DVE evacuations/normalization;
GPSIMD causal mask + partial-output stores; SP x-loads + V transposes.
"""

import numpy as np
import ml_dtypes

import concourse.bass as bass
import concourse.mybir as mybir
from concourse import bacc
from concourse.tile import TileContext

# Walrus's redundant-LDWEIGHTS elimination is off by default; enabling it
# measures ~12% faster end-to-end on this kernel with identical results.
import concourse.bass_utils as _bu

if not getattr(_bu, "_ldw_opt_patched", False):
    _orig_run_command = _bu.run_command

    def _run_command_ldw_opt(argv, **kwargs):
        argv = ["--enable-ldw-opt=true" if a == "--enable-ldw-opt=false"
                else a for a in argv]
        return _orig_run_command(argv, **kwargs)

    _bu.run_command = _run_command_ldw_opt
    _bu._ldw_opt_patched = True

F32 = mybir.dt.float32
BF16 = mybir.dt.bfloat16

B, T, C, H = 4, 2048, 1024, 16
HD = 64
NCORES = 8
CT = C // 128          # 8 C-tiles (contraction)
QT = 512               # q tile (free dim of S^T matmuls)
KT = 128               # k tile (partition dim of S^T)
VW = 130               # v subtile width: [A(64)|onesA|B(64)|onesB]
SCALE = 1.0 / np.sqrt(HD)

_CACHED = {}


def build_kernel(b=B, t=T):
    """Build the per-core SPMD program. t must be a multiple of 512."""
    assert t % QT == 0
    nq = t // QT           # q-tiles per sequence
    nst = t // 128         # 128-token subtiles per sequence
    bt = b * t

    nc = bacc.Bacc("TRN2", target_bir_lowering=False, debug=False,
                   num_devices=NCORES)

    xT = nc.dram_tensor("xT", [C, bt], BF16, kind="ExternalInput")
    # wq/wk/wv arrive pre-arranged in lhsT layout: [p, ct*128+m] = W[ct*128+p, m]
    wq = nc.dram_tensor("wq", [128, C], BF16, kind="ExternalInput")
    wk = nc.dram_tensor("wk", [128, C], BF16, kind="ExternalInput")
    wv = nc.dram_tensor("wv", [128, C], BF16, kind="ExternalInput")
    wp = nc.dram_tensor("wp", [128, C], BF16, kind="ExternalInput")
    bq = nc.dram_tensor("bq", [128, 1], F32, kind="ExternalInput")
    bk = nc.dram_tensor("bk", [128, 1], F32, kind="ExternalInput")
    out = nc.dram_tensor("out", [bt, C], BF16, kind="ExternalOutput")

    with TileContext(nc) as tc:
        with (
            tc.tile_pool(name="const", bufs=1) as constp,
            tc.tile_pool(name="xin", bufs=2 * CT) as xin,
            tc.tile_pool(name="qk", bufs=2) as qkp,
            tc.tile_pool(name="vst", bufs=2) as vstp,
            tc.tile_pool(name="vnat", bufs=3) as vnatp,
            tc.tile_pool(name="es", bufs=4) as esp,
            tc.tile_pool(name="yt", bufs=2) as ytp,
            tc.tile_pool(name="small", bufs=2) as smallp,
            tc.tile_pool(name="outsb", bufs=3) as outp,
            tc.tile_pool(name="ps_s", bufs=2, space="PSUM") as ps_s,
            tc.tile_pool(name="ps_ya", bufs=2, space="PSUM") as ps_ya,
            tc.tile_pool(name="ps_misc", bufs=2, space="PSUM") as ps_misc,
        ):
            # ---- constants / weights ----
            wq_sb = constp.tile([128, C], BF16, tag="wq")
            wk_sb = constp.tile([128, C], BF16, tag="wk")
            wv_sb = constp.tile([128, C], BF16, tag="wv")
            wp_sb = constp.tile([128, C], BF16, tag="wp")
            for w_dram, w_sb in ((wk, wk_sb), (wq, wq_sb), (wv, wv_sb),
                                 (wp, wp_sb)):
                nc.sync.dma_start(out=w_sb[:], in_=w_dram[:])
            bq_sb = constp.tile([128, 1], F32, tag="bq")
            bk_sb = constp.tile([128, 1], F32, tag="bk")
            nc.sync.dma_start(out=bq_sb[:], in_=bq[:])
            nc.sync.dma_start(out=bk_sb[:], in_=bk[:])

            one_f32 = constp.tile([128, 1], F32, tag="one")
            nc.vector.memset(one_f32[:], 1.0)

            def emit_proj_qt(pbi, yT_tile, pqt):
                for sj in range(QT // 128):
                    st = pqt * (QT // 128) + sj
                    osb = outp.tile([128, C], BF16, tag="osb")
                    for n in range(C // QT):
                        pp = ps_misc.tile([128, QT], F32, tag="m")
                        nc.tensor.matmul(
                            pp[:],
                            yT_tile[:, st * 128:(st + 1) * 128],
                            wp_sb[:, n * QT:(n + 1) * QT],
                            start=True, stop=True)
                        nc.vector.tensor_copy(
                            out=osb[:, n * QT:(n + 1) * QT], in_=pp[:])
                    nc.gpsimd.dma_start(
                        out=out[pbi * t + st * 128:
                                pbi * t + (st + 1) * 128, :],
                        in_=osb[:])

            def emit_x_loads(bi, xts):
                for colt in range(t // QT):
                    for ct in range(CT):
                        nc.sync.dma_start(
                            out=xts[ct][:, colt * QT:(colt + 1) * QT],
                            in_=xT[ct * 128:(ct + 1) * 128,
                                   bi * t + colt * QT:
                                   bi * t + (colt + 1) * QT],
                        )

            xts_next = [xin.tile([128, t], BF16, tag="xt", name=f"xt{ct}")
                        for ct in range(CT)]
            emit_x_loads(0, xts_next)

            for bi in range(b):
                xts = xts_next
                # v_sb per 128-token subtile: [A(64)|onesA(1)|B(64)|onesB(1)]
                v_sb = qkp.tile([128, nst * VW], BF16, tag="v")
                v_view = v_sb[:].rearrange("p (s w) -> p s w", w=VW)
                nc.vector.tensor_copy(
                    out=v_view[:, :, 64:65],
                    in_=one_f32[:].to_broadcast((128, nst, 1)))
                nc.vector.tensor_copy(
                    out=v_view[:, :, 129:130],
                    in_=one_f32[:].to_broadcast((128, nst, 1)))
                qT_sb = qkp.tile([128, t], BF16, tag="qT")
                kT_sb = qkp.tile([128, t], BF16, tag="kT")
                yT_sb = ytp.tile([128, t], BF16, tag="yT")

                # ================= QKV projection =================
                for colt in range(t // QT):
                    csl = slice(colt * QT, (colt + 1) * QT)
                    for w_sb, dst, bias in (
                        (wk_sb, kT_sb, bk_sb), (wq_sb, qT_sb, bq_sb),
                    ):
                        ps = ps_misc.tile([128, QT], F32, tag="m")
                        for ct in range(CT):
                            nc.tensor.matmul(
                                ps[:],
                                w_sb[:, ct * 128:(ct + 1) * 128],
                                xts[ct][:, csl],
                                start=(ct == 0), stop=(ct == CT - 1),
                            )
                        nc.vector.tensor_scalar_add(
                            out=dst[:, csl], in0=ps[:], scalar1=bias[:])
                    # V^T for this col tile; XBAR-transpose to natural layout
                    ps = ps_misc.tile([128, QT], F32, tag="m")
                    for ct in range(CT):
                        nc.tensor.matmul(
                            ps[:], wv_sb[:, ct * 128:(ct + 1) * 128],
                            xts[ct][:, csl],
                            start=(ct == 0), stop=(ct == CT - 1))
                    vstage = vstp.tile([128, QT], BF16, tag="vstage")
                    nc.vector.tensor_copy(out=vstage[:], in_=ps[:])
                    for sj in range(QT // 128):
                        st = colt * (QT // 128) + sj
                        vnat = vnatp.tile([128, 128], BF16, tag="vnat")
                        nc.sync.dma_start_transpose(
                            vnat[:], vstage[:, sj * 128:(sj + 1) * 128])
                        # one strided copy drops both head halves into place
                        dst3 = v_sb[:, st * VW:st * VW + VW].rearrange(
                            "p (h x) -> p h x", h=2)[:, :, 0:64]
                        src3 = vnat[:].rearrange("p (h x) -> p h x", h=2)
                        nc.gpsimd.tensor_copy(out=dst3, in_=src3)

                # prefetch next batch's x while attention below runs
                if bi + 1 < b:
                    xts_next = [xin.tile([128, t], BF16, tag="xt",
                                         name=f"xt{bi + 1}_{ct}")
                                for ct in range(CT)]
                    emit_x_loads(bi + 1, xts_next)

                # ================= attention (heads row-paired) =================
                for qt in range(nq):
                    n_k = (qt + 1) * (QT // KT)   # k-tiles of 128
                    q0 = qt * QT
                    yas = [ps_ya.tile([65, QT], F32, tag="ya",
                                      name=f"ya{_h}")
                           for _h in range(2)]
                    for kt in range(n_k):
                        lo = max(0, kt * KT - q0)
                        # S^T for both heads in one array pass: head A on PE
                        # rows 0-63, head B on rows 64-127 (row tiling).
                        sg = ps_s.tile([128, 2 * QT], F32, tag="sg")
                        es = esp.tile([128, 2 * QT], BF16, tag="es")
                        for h in range(2):
                            hsl = slice(h * 64, (h + 1) * 64)
                            nc.tensor.matmul(
                                sg[:, h * QT + lo:(h + 1) * QT],
                                kT_sb[hsl, kt * KT:(kt + 1) * KT],
                                qT_sb[hsl, q0 + lo:q0 + QT],
                                start=True, stop=True,
                            )
                        sg_v = sg[:].rearrange("p (h q) -> p h q", h=2)
                        es_v = es[:].rearrange("p (h q) -> p h q", h=2)
                        nc.scalar.activation(
                            es_v[:, :, lo:], sg_v[:, :, lo:],
                            mybir.ActivationFunctionType.Exp, scale=SCALE)
                        if kt * KT >= q0:
                            # causal band select, both heads in one op
                            nc.gpsimd.affine_select(
                                out=es_v[:, :, lo:lo + KT],
                                in_=es_v[:, :, lo:lo + KT],
                                compare_op=mybir.AluOpType.is_ge,
                                fill=0.0,
                                base=0,
                                channel_multiplier=-1,
                                pattern=[[0, 2], [1, KT]],
                            )
                        for h in range(2):
                            nc.tensor.matmul(
                                yas[h][:, lo:QT],
                                v_sb[:, kt * VW + 65 * h:
                                     kt * VW + 65 * h + 65],
                                es[:, h * QT + lo:(h + 1) * QT],
                                start=(kt == 0), stop=(kt == n_k - 1),
                            )
                    for h in range(2):
                        ya = yas[h]
                        # reciprocal can't read PSUM; stage the d-row first
                        srow = smallp.tile([1, QT], F32, tag="srow")
                        nc.vector.tensor_copy(
                            out=srow[0:1, :], in_=ya[64:65, :])
                        rr = smallp.tile([1, QT], F32, tag="rr")
                        nc.vector.reciprocal_approx_fast(
                            out=rr[0:1, :], in_=srow[0:1, :])
                        # broadcast recip across 64 partitions
                        bc = smallp.tile([64, QT], F32, tag="bc")
                        nc.gpsimd.partition_broadcast(
                            bc[:], rr[0:1, :], channels=64)
                        if h == 0:
                            nc.vector.tensor_mul(
                                out=yT_sb[0:64, q0:q0 + QT],
                                in0=ya[0:64, :], in1=bc[:])
                        else:
                            ytb = smallp.tile([64, QT], BF16, tag="ytb")
                            nc.vector.tensor_mul(
                                out=ytb[:], in0=ya[0:64, :], in1=bc[:])
                            nc.sync.dma_start(
                                out=yT_sb[64:128, q0:q0 + QT], in_=ytb[:])
                    # output projection for this q-tile, emitted immediately
                    emit_proj_qt(bi, yT_sb, qt)

    nc.compile()
    return nc


def _prep_inputs(x, W_attn, b_attn, W_proj, b_proj, b, t):
    bf = ml_dtypes.bfloat16
    xT_full = np.ascontiguousarray(
        x.reshape(b * t, C).T).astype(bf)

    def lhsT(wcol):
        # [C, 128] -> lhsT layout [128, C]: [p, ct*128+m] = W[ct*128+p, m]
        return np.ascontiguousarray(
            wcol.reshape(CT, 128, 128).transpose(1, 0, 2).reshape(128, C)
        ).astype(bf)

    in_maps = []
    for c in range(NCORES):
        sl = slice(c * 128, (c + 1) * 128)
        in_maps.append({
            "xT": xT_full,
            "wq": lhsT(W_attn[:, 0:1024][:, sl]),
            "wk": lhsT(W_attn[:, 1024:2048][:, sl]),
            "wv": lhsT(W_attn[:, 2048:3072][:, sl]),
            "wp": np.ascontiguousarray(W_proj[sl, :]).astype(bf),
            "bq": np.ascontiguousarray(
                b_attn[0:1024][sl].reshape(128, 1)).astype(np.float32),
            "bk": np.ascontiguousarray(
                b_attn[1024:2048][sl].reshape(128, 1)).astype(np.float32),
        })
    return in_maps


def kernel(x, W_attn, b_attn, W_proj, b_proj, _trace=False):
    from concourse.bass_utils import run_bass_kernel_spmd

    x = np.asarray(x, dtype=np.float32)
    W_attn = np.asarray(W_attn, dtype=np.float32)
    b_attn = np.asarray(b_attn, dtype=np.float32)
    W_proj = np.asarray(W_proj, dtype=np.float32)
    b_proj = np.asarray(b_proj, dtype=np.float32)
    b, t, c = x.shape

    key = (b, t)
    if key not in _CACHED:
        _CACHED[key] = build_kernel(b, t)
    nc = _CACHED[key]

    in_maps = _prep_inputs(x, W_attn, b_attn, W_proj, b_proj, b, t)
    res = run_bass_kernel_spmd(
        nc, in_maps, core_ids=list(range(NCORES)), trace=_trace)

    acc = res.results[0]["out"].astype(np.float32)
    for r in res.results[1:]:
        acc = acc + r["out"].astype(np.float32)
    acc += b_attn[2048:3072] @ W_proj + b_proj
    out = acc.reshape(b, t, c)
    if _trace:
        kernel.last_result = res
    return out


# revision 14
# speedup vs baseline: 1.2276x; 1.2276x over previous
"""Causal self-attention Trainium2 kernel — tensor-parallel over heads on 8 NeuronCores.

Problem: B=4, T=2048, C=1024, H=16 heads (head_dim 64), fp32 in/out.
Sharding: 2 heads per core. Each core computes qkv projection for its head
columns, full causal attention for its heads, and a partial output
projection (its W_proj rows); partials are summed on host.

Datapath is bf16 (x, weights, q/k/v, attention weights, y, output partials)
with fp32 PSUM accumulation and fp32 softmax denominators; measured max rel
error ~3e-3 against an fp64 reference. bf16 halves HBM traffic and SBUF
footprint and enables XBAR DMA transpose for the V tiles (no PE transposes).
Engines: PE matmuls only; ACT exp only; DVE evacuations/normalization;
GPSIMD causal mask + partial-output stores; SP x-loads + V transposes.
"""

import numpy as np
import ml_dtypes

import concourse.bass as bass
import concourse.mybir as mybir
from concourse import bacc
from concourse.tile import TileContext
from concourse.masks import make_identity

# Walrus's redundant-LDWEIGHTS elimination is off by default; enabling it
# measures ~12% faster end-to-end on this kernel with identical results.
import concourse.bass_utils as _bu

if not getattr(_bu, "_ldw_opt_patched", False):
    _orig_run_command = _bu.run_command

    def _run_command_ldw_opt(argv, **kwargs):
        argv = ["--enable-ldw-opt=true" if a == "--enable-ldw-opt=false"
                else a for a in argv]
        return _orig_run_command(argv, **kwargs)

    _bu.run_command = _run_command_ldw_opt
    _bu._ldw_opt_patched = True

F32 = mybir.dt.float32
BF16 = mybir.dt.bfloat16

B, T, C, H = 4, 2048, 1024, 16
HD = 64
NCORES = 8
CT = C // 128          # 8 C-tiles (contraction)
QT = 512               # q tile (free dim of S^T matmuls)
KT = 128               # k tile (partition dim of S^T)
VW = 130               # v subtile width: [A(64)|onesA|B(64)|onesB]
SCALE = 1.0 / np.sqrt(HD)

_CACHED = {}


def build_kernel(b=B, t=T):
    """Build the per-core SPMD program. t must be a multiple of 512."""
    assert t % QT == 0
    nq = t // QT           # q-tiles per sequence
    nst = t // 128         # 128-token subtiles per sequence
    bt = b * t

    nc = bacc.Bacc("TRN2", target_bir_lowering=False, debug=False,
                   num_devices=NCORES)

    xT = nc.dram_tensor("xT", [C, bt], BF16, kind="ExternalInput")
    # wq/wk/wv arrive pre-arranged in lhsT layout: [p, ct*128+m] = W[ct*128+p, m]
    wq = nc.dram_tensor("wq", [128, C], BF16, kind="ExternalInput")
    wk = nc.dram_tensor("wk", [128, C], BF16, kind="ExternalInput")
    wv = nc.dram_tensor("wv", [128, C], BF16, kind="ExternalInput")
    wp = nc.dram_tensor("wp", [128, C], BF16, kind="ExternalInput")
    bq = nc.dram_tensor("bq", [128, 1], F32, kind="ExternalInput")
    bk = nc.dram_tensor("bk", [128, 1], F32, kind="ExternalInput")
    out = nc.dram_tensor("out", [bt, C], BF16, kind="ExternalOutput")

    with TileContext(nc) as tc:
        with (
            tc.tile_pool(name="const", bufs=1) as constp,
            tc.tile_pool(name="xin", bufs=2 * CT) as xin,
            tc.tile_pool(name="qk", bufs=2) as qkp,
            tc.tile_pool(name="vst", bufs=2) as vstp,
            tc.tile_pool(name="es", bufs=4) as esp,
            tc.tile_pool(name="yt", bufs=2) as ytp,
            tc.tile_pool(name="small", bufs=2) as smallp,
            tc.tile_pool(name="outsb", bufs=3) as outp,
            tc.tile_pool(name="ps_s", bufs=2, space="PSUM") as ps_s,
            tc.tile_pool(name="ps_ya", bufs=2, space="PSUM") as ps_ya,
            tc.tile_pool(name="ps_misc", bufs=2, space="PSUM") as ps_misc,
        ):
            # ---- constants / weights ----
            ident = constp.tile([128, 128], BF16, tag="ident")
            make_identity(nc, ident[:])
            wq_sb = constp.tile([128, C], BF16, tag="wq")
            wk_sb = constp.tile([128, C], BF16, tag="wk")
            wv_sb = constp.tile([128, C], BF16, tag="wv")
            wp_sb = constp.tile([128, C], BF16, tag="wp")
            for w_dram, w_sb in ((wk, wk_sb), (wq, wq_sb), (wv, wv_sb),
                                 (wp, wp_sb)):
                nc.sync.dma_start(out=w_sb[:], in_=w_dram[:])
            bq_sb = constp.tile([128, 1], F32, tag="bq")
            bk_sb = constp.tile([128, 1], F32, tag="bk")
            nc.sync.dma_start(out=bq_sb[:], in_=bq[:])
            nc.sync.dma_start(out=bk_sb[:], in_=bk[:])

            one_f32 = constp.tile([128, 1], F32, tag="one")
            nc.vector.memset(one_f32[:], 1.0)

            def emit_proj_qt(pbi, yT_tile, pqt):
                for sj in range(QT // 128):
                    st = pqt * (QT // 128) + sj
                    osb = outp.tile([128, C], BF16, tag="osb")
                    for n in range(C // QT):
                        pp = ps_misc.tile([128, QT], F32, tag="m")
                        nc.tensor.matmul(
                            pp[:],
                            yT_tile[:, st * 128:(st + 1) * 128],
                            wp_sb[:, n * QT:(n + 1) * QT],
                            start=True, stop=True)
                        nc.vector.tensor_copy(
                            out=osb[:, n * QT:(n + 1) * QT], in_=pp[:])
                    nc.gpsimd.dma_start(
                        out=out[pbi * t + st * 128:
                                pbi * t + (st + 1) * 128, :],
                        in_=osb[:])

            def emit_x_loads(bi, xts):
                for colt in range(t // QT):
                    for ct in range(CT):
                        nc.sync.dma_start(
                            out=xts[ct][:, colt * QT:(colt + 1) * QT],
                            in_=xT[ct * 128:(ct + 1) * 128,
                                   bi * t + colt * QT:
                                   bi * t + (colt + 1) * QT],
                        )

            xts_next = [xin.tile([128, t], BF16, tag="xt", name=f"xt{ct}")
                        for ct in range(CT)]
            emit_x_loads(0, xts_next)

            for bi in range(b):
                xts = xts_next
                # v_sb per 128-token subtile: [A(64)|onesA(1)|B(64)|onesB(1)]
                v_sb = qkp.tile([128, nst * VW], BF16, tag="v")
                v_view = v_sb[:].rearrange("p (s w) -> p s w", w=VW)
                nc.vector.tensor_copy(
                    out=v_view[:, :, 64:65],
                    in_=one_f32[:].to_broadcast((128, nst, 1)))
                nc.vector.tensor_copy(
                    out=v_view[:, :, 129:130],
                    in_=one_f32[:].to_broadcast((128, nst, 1)))
                qT_sb = qkp.tile([128, t], BF16, tag="qT")
                kT_sb = qkp.tile([128, t], BF16, tag="kT")
                yT_sb = ytp.tile([128, t], BF16, tag="yT")

                # ================= QKV projection =================
                for colt in range(t // QT):
                    csl = slice(colt * QT, (colt + 1) * QT)
                    for w_sb, dst, bias in (
                        (wk_sb, kT_sb, bk_sb), (wq_sb, qT_sb, bq_sb),
                    ):
                        ps = ps_misc.tile([128, QT], F32, tag="m")
                        for ct in range(CT):
                            nc.tensor.matmul(
                                ps[:],
                                w_sb[:, ct * 128:(ct + 1) * 128],
                                xts[ct][:, csl],
                                start=(ct == 0), stop=(ct == CT - 1),
                            )
                        nc.scalar.activation(
                            out=dst[:, csl], in_=ps[:],
                            func=mybir.ActivationFunctionType.Identity,
                            bias=bias[:])
                    # V^T for this col tile; XBAR-transpose to natural layout
                    ps = ps_misc.tile([128, QT], F32, tag="m")
                    for ct in range(CT):
                        nc.tensor.matmul(
                            ps[:], wv_sb[:, ct * 128:(ct + 1) * 128],
                            xts[ct][:, csl],
                            start=(ct == 0), stop=(ct == CT - 1))
                    vstage = vstp.tile([128, QT], BF16, tag="vstage")
                    nc.vector.tensor_copy(out=vstage[:], in_=ps[:])
                    for sj in range(QT // 128):
                        st = colt * (QT // 128) + sj
                        vt_ps = ps_misc.tile([128, 128], BF16, tag="m")
                        nc.tensor.transpose(
                            vt_ps[:], vstage[:, sj * 128:(sj + 1) * 128],
                            ident[:])
                        # one strided copy drops both head halves into place
                        dst3 = v_sb[:, st * VW:st * VW + VW].rearrange(
                            "p (h x) -> p h x", h=2)[:, :, 0:64]
                        src3 = vt_ps[:].rearrange("p (h x) -> p h x", h=2)
                        nc.vector.tensor_copy(out=dst3, in_=src3)

                # prefetch next batch's x while attention below runs
                if bi + 1 < b:
                    xts_next = [xin.tile([128, t], BF16, tag="xt",
                                         name=f"xt{bi + 1}_{ct}")
                                for ct in range(CT)]
                    emit_x_loads(bi + 1, xts_next)

                # ================= attention (heads row-paired) =================
                for qt in range(nq):
                    n_k = (qt + 1) * (QT // KT)   # k-tiles of 128
                    q0 = qt * QT
                    yas = [ps_ya.tile([65, QT], F32, tag="ya",
                                      name=f"ya{_h}")
                           for _h in range(2)]
                    for kt in range(n_k):
                        lo = max(0, kt * KT - q0)
                        # S^T for both heads in one array pass: head A on PE
                        # rows 0-63, head B on rows 64-127 (row tiling).
                        sg = ps_s.tile([128, 2 * QT], F32, tag="sg")
                        es = esp.tile([128, 2 * QT], BF16, tag="es")
                        for h in range(2):
                            hsl = slice(h * 64, (h + 1) * 64)
                            nc.tensor.matmul(
                                sg[:, h * QT + lo:(h + 1) * QT],
                                kT_sb[hsl, kt * KT:(kt + 1) * KT],
                                qT_sb[hsl, q0 + lo:q0 + QT],
                                start=True, stop=True,
                            )
                        sg_v = sg[:].rearrange("p (h q) -> p h q", h=2)
                        es_v = es[:].rearrange("p (h q) -> p h q", h=2)
                        nc.scalar.activation(
                            es_v[:, :, lo:], sg_v[:, :, lo:],
                            mybir.ActivationFunctionType.Exp, scale=SCALE)
                        if kt * KT >= q0:
                            # causal band select, both heads in one op
                            nc.gpsimd.affine_select(
                                out=es_v[:, :, lo:lo + KT],
                                in_=es_v[:, :, lo:lo + KT],
                                compare_op=mybir.AluOpType.is_ge,
                                fill=0.0,
                                base=0,
                                channel_multiplier=-1,
                                pattern=[[0, 2], [1, KT]],
                            )
                        for h in range(2):
                            nc.tensor.matmul(
                                yas[h][:, lo:QT],
                                v_sb[:, kt * VW + 65 * h:
                                     kt * VW + 65 * h + 65],
                                es[:, h * QT + lo:(h + 1) * QT],
                                start=(kt == 0), stop=(kt == n_k - 1),
                            )
                    for h in range(2):
                        ya = yas[h]
                        # reciprocal can't read PSUM; stage the d-row first
                        srow = smallp.tile([1, QT], F32, tag="srow")
                        nc.vector.tensor_copy(
                            out=srow[0:1, :], in_=ya[64:65, :])
                        rr = smallp.tile([1, QT], F32, tag="rr")
                        nc.vector.reciprocal_approx_fast(
                            out=rr[0:1, :], in_=srow[0:1, :])
                        # broadcast recip across 64 partitions
                        bc = smallp.tile([64, QT], F32, tag="bc")
                        nc.gpsimd.partition_broadcast(
                            bc[:], rr[0:1, :], channels=64)
                        if h == 0:
                            nc.vector.tensor_mul(
                                out=yT_sb[0:64, q0:q0 + QT],
                                in0=ya[0:64, :], in1=bc[:])
                        else:
                            ytb = smallp.tile([64, QT], BF16, tag="ytb")
                            nc.vector.tensor_mul(
                                out=ytb[:], in0=ya[0:64, :], in1=bc[:])
                            nc.sync.dma_start(
                                out=yT_sb[64:128, q0:q0 + QT], in_=ytb[:])
                    # output projection for this q-tile, emitted immediately
                    emit_proj_qt(bi, yT_sb, qt)

    nc.compile()
    return nc


def _prep_inputs(x, W_attn, b_attn, W_proj, b_proj, b, t):
    bf = ml_dtypes.bfloat16
    xT_full = np.ascontiguousarray(
        x.reshape(b * t, C).T).astype(bf)

    def lhsT(wcol):
        # [C, 128] -> lhsT layout [128, C]: [p, ct*128+m] = W[ct*128+p, m]
        return np.ascontiguousarray(
            wcol.reshape(CT, 128, 128).transpose(1, 0, 2).reshape(128, C)
        ).astype(bf)

    in_maps = []
    for c in range(NCORES):
        sl = slice(c * 128, (c + 1) * 128)
        in_maps.append({
            "xT": xT_full,
            "wq": lhsT(W_attn[:, 0:1024][:, sl]),
            "wk": lhsT(W_attn[:, 1024:2048][:, sl]),
            "wv": lhsT(W_attn[:, 2048:3072][:, sl]),
            "wp": np.ascontiguousarray(W_proj[sl, :]).astype(bf),
            "bq": np.ascontiguousarray(
                b_attn[0:1024][sl].reshape(128, 1)).astype(np.float32),
            "bk": np.ascontiguousarray(
                b_attn[1024:2048][sl].reshape(128, 1)).astype(np.float32),
        })
    return in_maps


def kernel(x, W_attn, b_attn, W_proj, b_proj, _trace=False):
    from concourse.bass_utils import run_bass_kernel_spmd

    x = np.asarray(x, dtype=np.float32)
    W_attn = np.asarray(W_attn, dtype=np.float32)
    b_attn = np.asarray(b_attn, dtype=np.float32)
    W_proj = np.asarray(W_proj, dtype=np.float32)
    b_proj = np.asarray(b_proj, dtype=np.float32)
    b, t, c = x.shape

    key = (b, t)
    if key not in _CACHED:
        _CACHED[key] = build_kernel(b, t)
    nc = _CACHED[key]

    in_maps = _prep_inputs(x, W_attn, b_attn, W_proj, b_proj, b, t)
    res = run_bass_kernel_spmd(
        nc, in_maps, core_ids=list(range(NCORES)), trace=_trace)

    acc = res.results[0]["out"].astype(np.float32)
    for r in res.results[1:]:
        acc = acc + r["out"].astype(np.float32)
    acc += b_attn[2048:3072] @ W_proj + b_proj
    out = acc.reshape(b, t, c)
    if _trace:
        kernel.last_result = res
    return out


# revision 20
# speedup vs baseline: 1.3465x; 1.0968x over previous
"""Causal self-attention Trainium2 kernel — tensor-parallel over heads on 8 NeuronCores.

Problem: B=4, T=2048, C=1024, H=16 heads (head_dim 64), fp32 in/out.
Sharding: 2 heads per core. Each core computes qkv projection for its head
columns, full causal attention for its heads, and a partial output
projection (its W_proj rows); partials are summed on host.

Datapath is bf16 (x, weights, q/k/v, attention weights, y, output partials)
with fp32 PSUM accumulation and fp32 softmax denominators; measured max rel
error ~3e-3 against an fp64 reference. bf16 halves HBM traffic and SBUF
footprint and enables XBAR DMA transpose for the V tiles (no PE transposes).
Engines: PE matmuls only; ACT exp only; DVE evacuations/normalization;
GPSIMD causal mask + partial-output stores; SP x-loads + V transposes.
"""

import numpy as np
import ml_dtypes

import concourse.bass as bass
import concourse.mybir as mybir
from concourse import bacc
from concourse.tile import TileContext
from concourse.masks import make_identity

# Walrus's redundant-LDWEIGHTS elimination is off by default; enabling it
# measures ~12% faster end-to-end on this kernel with identical results.
import concourse.bass_utils as _bu

if not getattr(_bu, "_ldw_opt_patched", False):
    _orig_run_command = _bu.run_command

    def _run_command_ldw_opt(argv, **kwargs):
        argv = ["--enable-ldw-opt=true" if a == "--enable-ldw-opt=false"
                else a for a in argv]
        return _orig_run_command(argv, **kwargs)

    _bu.run_command = _run_command_ldw_opt
    _bu._ldw_opt_patched = True

F32 = mybir.dt.float32
BF16 = mybir.dt.bfloat16

B, T, C, H = 4, 2048, 1024, 16
HD = 64
NCORES = 8
CT = C // 128          # 8 C-tiles (contraction)
QT = 512               # q tile (free dim of S^T matmuls)
KT = 128               # k tile (partition dim of S^T)
VW = 130               # v subtile width: [A(64)|onesA|B(64)|onesB]
SCALE = 1.0 / np.sqrt(HD)

_CACHED = {}


def build_kernel(b=B, t=T):
    """Build the per-core SPMD program. t must be a multiple of 512."""
    assert t % QT == 0
    nq = t // QT           # q-tiles per sequence
    nst = t // 128         # 128-token subtiles per sequence
    bt = b * t

    nc = bacc.Bacc("TRN2", target_bir_lowering=False, debug=False,
                   num_devices=NCORES)

    xT = nc.dram_tensor("xT", [C, bt], BF16, kind="ExternalInput")
    # wq/wk/wv arrive pre-arranged in lhsT layout: [p, ct*128+m] = W[ct*128+p, m]
    wq = nc.dram_tensor("wq", [128, C], BF16, kind="ExternalInput")
    wk = nc.dram_tensor("wk", [128, C], BF16, kind="ExternalInput")
    wv = nc.dram_tensor("wv", [128, C], BF16, kind="ExternalInput")
    wp = nc.dram_tensor("wp", [128, C], BF16, kind="ExternalInput")
    bq = nc.dram_tensor("bq", [128, 1], F32, kind="ExternalInput")
    bk = nc.dram_tensor("bk", [128, 1], F32, kind="ExternalInput")
    out = nc.dram_tensor("out", [bt, C], BF16, kind="ExternalOutput")

    with TileContext(nc) as tc:
        with (
            tc.tile_pool(name="const", bufs=1) as constp,
            tc.tile_pool(name="xin", bufs=2 * CT) as xin,
            tc.tile_pool(name="qk", bufs=2) as qkp,
            tc.tile_pool(name="vst", bufs=2) as vstp,
            tc.tile_pool(name="es", bufs=4) as esp,
            tc.tile_pool(name="yt", bufs=2) as ytp,
            tc.tile_pool(name="small", bufs=2) as smallp,
            tc.tile_pool(name="outsb", bufs=3) as outp,
            tc.tile_pool(name="ps_s", bufs=2, space="PSUM") as ps_s,
            tc.tile_pool(name="ps_ya", bufs=2, space="PSUM") as ps_ya,
            tc.tile_pool(name="ps_misc", bufs=2, space="PSUM") as ps_misc,
        ):
            # ---- constants / weights ----
            ident = constp.tile([128, 128], BF16, tag="ident")
            make_identity(nc, ident[:])
            wq_sb = constp.tile([128, C], BF16, tag="wq")
            wk_sb = constp.tile([128, C], BF16, tag="wk")
            wv_sb = constp.tile([128, C], BF16, tag="wv")
            wp_sb = constp.tile([128, C], BF16, tag="wp")
            for w_dram, w_sb in ((wk, wk_sb), (wq, wq_sb), (wv, wv_sb),
                                 (wp, wp_sb)):
                nc.sync.dma_start(out=w_sb[:], in_=w_dram[:])
            bq_sb = constp.tile([128, 1], F32, tag="bq")
            bk_sb = constp.tile([128, 1], F32, tag="bk")
            nc.sync.dma_start(out=bq_sb[:], in_=bq[:])
            nc.sync.dma_start(out=bk_sb[:], in_=bk[:])

            one_f32 = constp.tile([128, 1], F32, tag="one")
            nc.vector.memset(one_f32[:], 1.0)

            def emit_proj_qt(pbi, yT_tile, pqt):
                for sj in range(QT // 128):
                    st = pqt * (QT // 128) + sj
                    osb = outp.tile([128, C], BF16, tag="osb")
                    for n in range(C // QT):
                        pp = ps_misc.tile([128, QT], F32, tag="m")
                        nc.tensor.matmul(
                            pp[:],
                            yT_tile[:, st * 128:(st + 1) * 128],
                            wp_sb[:, n * QT:(n + 1) * QT],
                            start=True, stop=True)
                        nc.vector.tensor_copy(
                            out=osb[:, n * QT:(n + 1) * QT], in_=pp[:])
                    nc.gpsimd.dma_start(
                        out=out[pbi * t + st * 128:
                                pbi * t + (st + 1) * 128, :],
                        in_=osb[:])

            def emit_x_loads(bi, xts):
                for colt in range(t // QT):
                    for ct in range(CT):
                        nc.sync.dma_start(
                            out=xts[ct][:, colt * QT:(colt + 1) * QT],
                            in_=xT[ct * 128:(ct + 1) * 128,
                                   bi * t + colt * QT:
                                   bi * t + (colt + 1) * QT],
                        )

            xts_next = [xin.tile([128, t], BF16, tag="xt", name=f"xt{ct}")
                        for ct in range(CT)]
            emit_x_loads(0, xts_next)
            pending = None   # (bi, yT_tile, qt) awaiting projection

            for bi in range(b):
                xts = xts_next
                # v_sb per 128-token subtile: [A(64)|onesA(1)|B(64)|onesB(1)]
                v_sb = qkp.tile([128, nst * VW], BF16, tag="v")
                v_view = v_sb[:].rearrange("p (s w) -> p s w", w=VW)
                nc.vector.tensor_copy(
                    out=v_view[:, :, 64:65],
                    in_=one_f32[:].to_broadcast((128, nst, 1)))
                nc.vector.tensor_copy(
                    out=v_view[:, :, 129:130],
                    in_=one_f32[:].to_broadcast((128, nst, 1)))
                qT_sb = qkp.tile([128, t], BF16, tag="qT")
                kT_sb = qkp.tile([128, t], BF16, tag="kT")
                yT_sb = ytp.tile([128, t], BF16, tag="yT")

                # ================= QKV projection =================
                for colt in range(t // QT):
                    csl = slice(colt * QT, (colt + 1) * QT)
                    for w_sb, dst, bias in (
                        (wk_sb, kT_sb, bk_sb), (wq_sb, qT_sb, bq_sb),
                    ):
                        ps = ps_misc.tile([128, QT], F32, tag="m")
                        for ct in range(CT):
                            nc.tensor.matmul(
                                ps[:],
                                w_sb[:, ct * 128:(ct + 1) * 128],
                                xts[ct][:, csl],
                                start=(ct == 0), stop=(ct == CT - 1),
                            )
                        nc.scalar.activation(
                            out=dst[:, csl], in_=ps[:],
                            func=mybir.ActivationFunctionType.Identity,
                            bias=bias[:])
                    # V^T for this col tile; XBAR-transpose to natural layout
                    ps = ps_misc.tile([128, QT], F32, tag="m")
                    for ct in range(CT):
                        nc.tensor.matmul(
                            ps[:], wv_sb[:, ct * 128:(ct + 1) * 128],
                            xts[ct][:, csl],
                            start=(ct == 0), stop=(ct == CT - 1))
                    vstage = vstp.tile([128, QT], BF16, tag="vstage")
                    nc.vector.tensor_copy(out=vstage[:], in_=ps[:])
                    for sj in range(QT // 128):
                        st = colt * (QT // 128) + sj
                        vt_ps = ps_misc.tile([128, 128], BF16, tag="m")
                        nc.tensor.transpose(
                            vt_ps[:], vstage[:, sj * 128:(sj + 1) * 128],
                            ident[:])
                        # one strided copy drops both head halves into place
                        dst3 = v_sb[:, st * VW:st * VW + VW].rearrange(
                            "p (h x) -> p h x", h=2)[:, :, 0:64]
                        src3 = vt_ps[:].rearrange("p (h x) -> p h x", h=2)
                        nc.vector.tensor_copy(out=dst3, in_=src3)

                # prefetch next batch's x while attention below runs
                if bi + 1 < b:
                    xts_next = [xin.tile([128, t], BF16, tag="xt",
                                         name=f"xt{bi + 1}_{ct}")
                                for ct in range(CT)]
                    emit_x_loads(bi + 1, xts_next)

                # ================= attention (heads row-paired) =================
                for qt in range(nq):
                    n_k = (qt + 1) * (QT // KT)   # k-tiles of 128
                    q0 = qt * QT
                    yas = [ps_ya.tile([65, QT], F32, tag="ya",
                                      name=f"ya{_h}")
                           for _h in range(2)]
                    for kt in range(n_k):
                        if kt == 1 and pending is not None:
                            # one-qt-delayed projection: its inputs are ready
                            # long before the queue reaches it, so the PE
                            # never stalls on the normalization chain
                            emit_proj_qt(*pending)
                            pending = None
                        lo = max(0, kt * KT - q0)
                        # S^T for both heads in one array pass: head A on PE
                        # rows 0-63, head B on rows 64-127 (row tiling).
                        sg = ps_s.tile([128, 2 * QT], F32, tag="sg")
                        es = esp.tile([128, 2 * QT], BF16, tag="es")
                        for h in range(2):
                            hsl = slice(h * 64, (h + 1) * 64)
                            nc.tensor.matmul(
                                sg[:, h * QT + lo:(h + 1) * QT],
                                kT_sb[hsl, kt * KT:(kt + 1) * KT],
                                qT_sb[hsl, q0 + lo:q0 + QT],
                                start=True, stop=True,
                            )
                        sg_v = sg[:].rearrange("p (h q) -> p h q", h=2)
                        es_v = es[:].rearrange("p (h q) -> p h q", h=2)
                        nc.scalar.activation(
                            es_v[:, :, lo:], sg_v[:, :, lo:],
                            mybir.ActivationFunctionType.Exp, scale=SCALE)
                        if kt * KT >= q0:
                            # causal band select, both heads in one op
                            nc.gpsimd.affine_select(
                                out=es_v[:, :, lo:lo + KT],
                                in_=es_v[:, :, lo:lo + KT],
                                compare_op=mybir.AluOpType.is_ge,
                                fill=0.0,
                                base=0,
                                channel_multiplier=-1,
                                pattern=[[0, 2], [1, KT]],
                            )
                        for h in range(2):
                            nc.tensor.matmul(
                                yas[h][:, lo:QT],
                                v_sb[:, kt * VW + 65 * h:
                                     kt * VW + 65 * h + 65],
                                es[:, h * QT + lo:(h + 1) * QT],
                                start=(kt == 0), stop=(kt == n_k - 1),
                            )
                    for h in range(2):
                        ya = yas[h]
                        # evacuate fast so the next qt's AV can reuse the bank
                        ya_sb = smallp.tile([64, QT], F32, tag="yasb",
                                            bufs=4)
                        nc.vector.tensor_copy(out=ya_sb[:], in_=ya[0:64, :])
                        srow = smallp.tile([1, QT], F32, tag="srow")
                        nc.vector.tensor_copy(
                            out=srow[0:1, :], in_=ya[64:65, :])
                        rr = smallp.tile([1, QT], F32, tag="rr")
                        nc.vector.reciprocal_approx_fast(
                            out=rr[0:1, :], in_=srow[0:1, :])
                        # broadcast recip across 64 partitions
                        bc = smallp.tile([64, QT], F32, tag="bc")
                        nc.gpsimd.partition_broadcast(
                            bc[:], rr[0:1, :], channels=64)
                        if h == 0:
                            nc.vector.tensor_mul(
                                out=yT_sb[0:64, q0:q0 + QT],
                                in0=ya_sb[:], in1=bc[:])
                        else:
                            ytb = smallp.tile([64, QT], BF16, tag="ytb")
                            nc.vector.tensor_mul(
                                out=ytb[:], in0=ya_sb[:], in1=bc[:])
                            nc.sync.dma_start(
                                out=yT_sb[64:128, q0:q0 + QT], in_=ytb[:])
                    pending = (bi, yT_sb, qt)

            emit_proj_qt(*pending)

    nc.compile()
    return nc


def _prep_inputs(x, W_attn, b_attn, W_proj, b_proj, b, t):
    bf = ml_dtypes.bfloat16
    xT_full = np.ascontiguousarray(
        x.reshape(b * t, C).T).astype(bf)

    def lhsT(wcol):
        # [C, 128] -> lhsT layout [128, C]: [p, ct*128+m] = W[ct*128+p, m]
        return np.ascontiguousarray(
            wcol.reshape(CT, 128, 128).transpose(1, 0, 2).reshape(128, C)
        ).astype(bf)

    in_maps = []
    for c in range(NCORES):
        sl = slice(c * 128, (c + 1) * 128)
        in_maps.append({
            "xT": xT_full,
            "wq": lhsT(W_attn[:, 0:1024][:, sl]),
            "wk": lhsT(W_attn[:, 1024:2048][:, sl]),
            "wv": lhsT(W_attn[:, 2048:3072][:, sl]),
            "wp": np.ascontiguousarray(W_proj[sl, :]).astype(bf),
            "bq": np.ascontiguousarray(
                b_attn[0:1024][sl].reshape(128, 1)).astype(np.float32),
            "bk": np.ascontiguousarray(
                b_attn[1024:2048][sl].reshape(128, 1)).astype(np.float32),
        })
    return in_maps


def kernel(x, W_attn, b_attn, W_proj, b_proj, _trace=False):
    from concourse.bass_utils import run_bass_kernel_spmd

    x = np.asarray(x, dtype=np.float32)
    W_attn = np.asarray(W_attn, dtype=np.float32)
    b_attn = np.asarray(b_attn, dtype=np.float32)
    W_proj = np.asarray(W_proj, dtype=np.float32)
    b_proj = np.asarray(b_proj, dtype=np.float32)
    b, t, c = x.shape

    key = (b, t)
    if key not in _CACHED:
        _CACHED[key] = build_kernel(b, t)
    nc = _CACHED[key]

    in_maps = _prep_inputs(x, W_attn, b_attn, W_proj, b_proj, b, t)
    res = run_bass_kernel_spmd(
        nc, in_maps, core_ids=list(range(NCORES)), trace=_trace)

    acc = res.results[0]["out"].astype(np.float32)
    for r in res.results[1:]:
        acc = acc + r["out"].astype(np.float32)
    acc += b_attn[2048:3072] @ W_proj + b_proj
    out = acc.reshape(b, t, c)
    if _trace:
        kernel.last_result = res
    return out
